# revision 3
# baseline (speedup 1.0000x reference)
"""AFT-Full (Attention Free Transformer) forward on 8 Trainium2 NeuronCores.

Reference computation (B=4, T=2048, D=1024, H=16, dh=64):
    Q = x @ Wq.T + bq ; K = x @ Wk.T + bk ; V = x @ Wv.T + bv
    ew = exp(wbias)                       # [T, T]
    numer = ew @ (exp(K) * V)             # per (b, h)
    denom = ew @ exp(K)
    out = sigmoid(Q) * numer / denom
    y = out @ Wo.T + bo

Sharding: 8 cores = 4 batches x 2 output-row halves. Core c handles batch
b = c//2 and output rows [th*1024, th*1024+1024) with th = c%2. K/V are
(re)computed for the full sequence on both cores of a batch pair; no
cross-core communication is needed at all.

Identities used to avoid broadcast-bias work on device:
  - bk cancels exactly in numer/denom (exp(K+bk) = exp(K)*exp(bk), and the
    exp(bk[d]) factor is constant over the contraction index s).
  - bv shifts the ratio: ew@(eK*(V+bv)) / ew@eK = ew@(eK*V)/ew@eK + bv[d],
    applied as a per-partition scalar add in the [d, t] layout.
  - bq is a per-partition bias in the Q^T [e, t] layout, fused into the
    sigmoid activation. bo is added during PSUM evacuation from a
    host-broadcast [128, D] tile.

All matmuls run in bf16 (fp32 PSUM accumulation). The host pre-transposes
x, the weights and wbias so the device needs no transposes, and rolls the
contraction (s) axis by the core's t-offset so a single shared program
serves both t-halves.
"""

import sys

if "/opt/trn_rl_repo" not in sys.path:
    sys.path.insert(0, "/opt/trn_rl_repo")

import numpy as np
import ml_dtypes

BF16 = ml_dtypes.bfloat16

B, T, D = 4, 2048, 1024
TH = T // 2  # rows per core
P = 128
CH = 512  # psum chunk (one fp32 bank)
DT = D // P  # 8   d-tiles
ST = T // P  # 16  s-tiles
NCH_D = D // CH  # 2
NCH_T = TH // CH  # 2

_cache = {}


def _build_nc():
    import concourse.mybir as mybir
    import concourse.tile as tile
    from concourse import bacc

    dt = mybir.dt
    BF = dt.bfloat16
    F32 = dt.float32
    Act = mybir.ActivationFunctionType
    Alu = mybir.AluOpType

    nc = bacc.Bacc("TRN2")

    xT = nc.dram_tensor("xT", [D, T], BF, kind="ExternalInput")
    wqT = nc.dram_tensor("wqT", [D, D], BF, kind="ExternalInput")
    wkT = nc.dram_tensor("wkT", [D, D], BF, kind="ExternalInput")
    wvT = nc.dram_tensor("wvT", [D, D], BF, kind="ExternalInput")
    woT = nc.dram_tensor("woT", [D, D], BF, kind="ExternalInput")
    wbT = nc.dram_tensor("wbT", [T, TH], BF, kind="ExternalInput")
    bqc = nc.dram_tensor("bqc", [P, DT], F32, kind="ExternalInput")
    bvc = nc.dram_tensor("bvc", [P, DT], F32, kind="ExternalInput")
    bob = nc.dram_tensor("bob", [P, D], F32, kind="ExternalInput")
    y = nc.dram_tensor("y", [TH, D], F32, kind="ExternalOutput")

    xT_v = xT.rearrange("(o p) t -> p o t", p=P)
    wq_v = wqT.rearrange("(o p) e -> p o e", p=P)
    wk_v = wkT.rearrange("(o p) e -> p o e", p=P)
    wv_v = wvT.rearrange("(o p) e -> p o e", p=P)
    wo_v = woT.rearrange("(o p) e -> p o e", p=P)
    wb_v = wbT.rearrange("(o p) t -> p o t", p=P)
    y_v = y.rearrange("(o p) e -> p o e", p=P)

    with tile.TileContext(nc) as tc:
        with (
            tc.tile_pool(name="big", bufs=1) as big,
            tc.tile_pool(name="w", bufs=2) as wpool,
            tc.tile_pool(name="tmp", bufs=3) as tmp,
            tc.tile_pool(name="bias", bufs=1) as biasp,
            tc.tile_pool(name="psum", bufs=6, space="PSUM") as psum,
        ):
            bq_s = biasp.tile([P, DT], F32, tag="bq")
            nc.sync.dma_start(bq_s[:], bqc[:])
            bv_s = biasp.tile([P, DT], F32, tag="bv")
            nc.sync.dma_start(bv_s[:], bvc[:])
            bo_s = biasp.tile([P, D], F32, tag="bo")
            nc.sync.dma_start(bo_s[:], bob[:])

            eK = big.tile([P, ST, D], BF, tag="eK")
            eKV = big.tile([P, ST, D], BF, tag="eKV")
            sigQT = big.tile([P, DT, TH], BF, tag="sigQT")
            outPT = big.tile([P, DT, TH], BF, tag="outPT")

            with tc.tile_pool(name="x", bufs=1) as xpool:
                xs = xpool.tile([P, DT, T], BF, tag="xs")
                wk_s = wpool.tile([P, DT, D], BF, tag="w")
                wv_s = wpool.tile([P, DT, D], BF, tag="w")
                # interleave the input DMAs in consumption order so the
                # first matmuls can start after ~1 MB instead of ~10 MB
                for k in range(DT):
                    nc.sync.dma_start(xs[:, k, :], xT_v[:, k, :])
                    nc.sync.dma_start(wk_s[:, k, :], wk_v[:, k, :])
                for k in range(DT):
                    nc.sync.dma_start(wv_s[:, k, :], wv_v[:, k, :])
                wq_s = wpool.tile([P, DT, D], BF, tag="w")
                for k in range(DT):
                    nc.sync.dma_start(wq_s[:, k, :], wq_v[:, k, :])

                # ---- K, V projections -> eK = exp(K), eKV = eK * V ----
                for st in range(ST):
                    tsl = slice(st * P, (st + 1) * P)
                    for ec in range(NCH_D):
                        esl = slice(ec * CH, (ec + 1) * CH)
                        pk = psum.tile([P, CH], F32, tag="ps")
                        for k in range(DT):
                            nc.tensor.matmul(
                                pk[:], xs[:, k, tsl], wk_s[:, k, esl],
                                start=(k == 0), stop=(k == DT - 1),
                            )
                        nc.scalar.activation(eK[:, st, esl], pk[:], Act.Exp)
                    for ec in range(NCH_D):
                        esl = slice(ec * CH, (ec + 1) * CH)
                        pv = psum.tile([P, CH], F32, tag="ps")
                        for k in range(DT):
                            nc.tensor.matmul(
                                pv[:], xs[:, k, tsl], wv_s[:, k, esl],
                                start=(k == 0), stop=(k == DT - 1),
                            )
                        nc.vector.tensor_tensor(
                            eKV[:, st, esl], eK[:, st, esl], pv[:], Alu.mult
                        )

                # ---- Q^T projection -> sigQT = sigmoid(Q^T + bq) ----
                # (columns 0:TH of xs are this core's own t rows)
                for et in range(DT):
                    esl = slice(et * P, (et + 1) * P)
                    for c in range(NCH_T):
                        tsl = slice(c * CH, (c + 1) * CH)
                        pq = psum.tile([P, CH], F32, tag="ps")
                        for k in range(DT):
                            nc.tensor.matmul(
                                pq[:], wq_s[:, k, esl], xs[:, k, tsl],
                                start=(k == 0), stop=(k == DT - 1),
                            )
                        nc.scalar.activation(
                            sigQT[:, et, tsl], pq[:], Act.Sigmoid,
                            bias=bq_s[:, et : et + 1],
                        )

            # ---- ew^T = exp(wbias^T), staged two s-tiles at a time ----
            with (
                tc.tile_pool(name="ew", bufs=1) as ewpool,
                tc.tile_pool(name="wbst", bufs=2) as wbpool,
            ):
                ewT = ewpool.tile([P, ST, TH], BF, tag="ewT")
                for q in range(8):
                    stg = wbpool.tile([P, 2, TH], BF, tag="wbst")
                    nc.sync.dma_start(stg[:], wb_v[:, q * 2 : (q + 1) * 2, :])
                    nc.scalar.activation(
                        ewT[:, q * 2 : (q + 1) * 2, :], stg[:], Act.Exp
                    )

                # ---- AFT: numerT/denomT accumulation + ratio ----
                for dti in range(DT):
                    dsl = slice(dti * P, (dti + 1) * P)
                    for c in range(NCH_T):
                        tsl = slice(c * CH, (c + 1) * CH)
                        pn = psum.tile([P, CH], F32, tag="ps")
                        pd = psum.tile([P, CH], F32, tag="ps")
                        for ss in range(ST):
                            nc.tensor.matmul(
                                pn[:], eKV[:, ss, dsl], ewT[:, ss, tsl],
                                start=(ss == 0), stop=(ss == ST - 1),
                            )
                        for ss in range(ST):
                            nc.tensor.matmul(
                                pd[:], eK[:, ss, dsl], ewT[:, ss, tsl],
                                start=(ss == 0), stop=(ss == ST - 1),
                            )
                        rec = tmp.tile([P, CH], F32, tag="rec")
                        nc.vector.reciprocal(rec[:], pd[:])
                        rat = tmp.tile([P, CH], F32, tag="rat")
                        nc.vector.tensor_tensor(rat[:], pn[:], rec[:], Alu.mult)
                        nc.vector.tensor_scalar(
                            rat[:], rat[:], bv_s[:, dti : dti + 1], None, Alu.add
                        )
                        nc.vector.tensor_tensor(
                            outPT[:, dti, tsl], rat[:], sigQT[:, dti, tsl],
                            Alu.mult,
                        )

                # ---- output projection: y = outPT^T @ woT + bo ----
                wo_s = wpool.tile([P, DT, D], BF, tag="w")
                for k in range(DT):
                    nc.sync.dma_start(wo_s[:, k, :], wo_v[:, k, :])
                for tt in range(DT):
                    tsl = slice(tt * P, (tt + 1) * P)
                    for ec in range(NCH_D):
                        esl = slice(ec * CH, (ec + 1) * CH)
                        py = psum.tile([P, CH], F32, tag="ps")
                        for k in range(DT):
                            nc.tensor.matmul(
                                py[:], outPT[:, k, tsl], wo_s[:, k, esl],
                                start=(k == 0), stop=(k == DT - 1),
                            )
                        ysb = tmp.tile([P, CH], F32, tag="ysb")
                        nc.vector.tensor_tensor(
                            ysb[:], py[:], bo_s[:, esl], Alu.add
                        )
                        nc.sync.dma_start(y_v[:, tt, esl], ysb[:])

    nc.compile()
    return nc


def _get_nc():
    if "nc" not in _cache:
        _cache["nc"] = _build_nc()
    return _cache["nc"]


def kernel(x, dummy, Wq, bq, Wk, bk, Wv, bv, Wo, bo, wbias):
    import os

    x = np.asarray(x, np.float32)
    Wq = np.asarray(Wq, np.float32)
    Wk = np.asarray(Wk, np.float32)
    Wv = np.asarray(Wv, np.float32)
    Wo = np.asarray(Wo, np.float32)
    bq = np.asarray(bq, np.float32)
    bv = np.asarray(bv, np.float32)
    bo = np.asarray(bo, np.float32)
    wbias = np.asarray(wbias, np.float32)

    wqT = np.ascontiguousarray(Wq.T).astype(BF16)  # [d_in, e_out]
    wkT = np.ascontiguousarray(Wk.T).astype(BF16)
    wvT = np.ascontiguousarray(Wv.T).astype(BF16)
    woT = np.ascontiguousarray(Wo.T).astype(BF16)
    bqc = np.ascontiguousarray(bq.reshape(DT, P).T)  # [P, DT]
    bvc = np.ascontiguousarray(bv.reshape(DT, P).T)
    bob = np.ascontiguousarray(np.broadcast_to(bo, (P, D)))

    in_maps = []
    for c in range(8):
        b, th = c // 2, c % 2
        t0 = th * TH
        xTb = x[b].T  # [D, T]
        # roll the s (contraction) axis so this core's own t rows come
        # first; Q then always reads columns [0, TH).
        xT_in = np.concatenate([xTb[:, t0:], xTb[:, :t0]], axis=1)
        wbn = wbias[t0 : t0 + TH, :].T  # [s, t] natural s order
        wb_in = np.concatenate([wbn[t0:, :], wbn[:t0, :]], axis=0)
        in_maps.append(
            {
                "xT": np.ascontiguousarray(xT_in).astype(BF16),
                "wqT": wqT,
                "wkT": wkT,
                "wvT": wvT,
                "woT": woT,
                "wbT": np.ascontiguousarray(wb_in).astype(BF16),
                "bqc": bqc,
                "bvc": bvc,
                "bob": bob,
            }
        )

    from concourse.bass_utils import run_bass_kernel_spmd

    nc = _get_nc()
    trace = bool(os.environ.get("AFT_TRACE"))
    res = run_bass_kernel_spmd(
        nc, in_maps, core_ids=list(range(8)), trace=trace
    )
    kernel._last_exec_ns = res.exec_time_ns
    kernel._last_result = res

    out = np.empty((B, T, D), np.float32)
    for c in range(8):
        b, th = c // 2, c % 2
        out[b, th * TH : (th + 1) * TH, :] = res.results[c]["y"]
    return out


# revision 6
# speedup vs baseline: 1.0284x; 1.0284x over previous
"""AFT-Full (Attention Free Transformer) forward on 8 Trainium2 NeuronCores.

Reference computation (B=4, T=2048, D=1024, H=16, dh=64):
    Q = x @ Wq.T + bq ; K = x @ Wk.T + bk ; V = x @ Wv.T + bv
    ew = exp(wbias)                       # [T, T]
    numer = ew @ (exp(K) * V)             # per (b, h)
    denom = ew @ exp(K)
    out = sigmoid(Q) * numer / denom
    y = out @ Wo.T + bo

Sharding: 8 cores = 4 batches x 2 output-row halves. Core c handles batch
b = c//2 and output rows [th*1024, th*1024+1024) with th = c%2. K/V are
(re)computed for the full sequence on both cores of a batch pair; no
cross-core communication is needed at all.

Identities used to avoid broadcast-bias work on device:
  - bk cancels exactly in numer/denom (exp(K+bk) = exp(K)*exp(bk), and the
    exp(bk[d]) factor is constant over the contraction index s).
  - bv shifts the ratio: ew@(eK*(V+bv)) / ew@eK = ew@(eK*V)/ew@eK + bv[d],
    applied as a per-partition scalar add in the [d, t] layout.
  - bq is a per-partition bias in the Q^T [e, t] layout, fused into the
    sigmoid activation. bo is added during PSUM evacuation from a
    host-broadcast [128, D] tile.

All matmuls run in bf16 (fp32 PSUM accumulation). The host pre-transposes
x, the weights and wbias so the device needs no transposes, and rolls the
contraction (s) axis by the core's t-offset so a single shared program
serves both t-halves.
"""

import sys

if "/opt/trn_rl_repo" not in sys.path:
    sys.path.insert(0, "/opt/trn_rl_repo")

import numpy as np
import ml_dtypes

BF16 = ml_dtypes.bfloat16

B, T, D = 4, 2048, 1024
TH = T // 2  # rows per core
P = 128
CH = 512  # psum chunk (one fp32 bank)
DT = D // P  # 8   d-tiles
ST = T // P  # 16  s-tiles
NCH_D = D // CH  # 2
NCH_T = TH // CH  # 2

_cache = {}


def _build_nc():
    import concourse.mybir as mybir
    import concourse.tile as tile
    from concourse import bacc

    dt = mybir.dt
    BF = dt.bfloat16
    F32 = dt.float32
    Act = mybir.ActivationFunctionType
    Alu = mybir.AluOpType

    nc = bacc.Bacc("TRN2")

    xT = nc.dram_tensor("xT", [D, T], BF, kind="ExternalInput")
    wqT = nc.dram_tensor("wqT", [D, D], BF, kind="ExternalInput")
    wkT = nc.dram_tensor("wkT", [D, D], BF, kind="ExternalInput")
    wvT = nc.dram_tensor("wvT", [D, D], BF, kind="ExternalInput")
    woT = nc.dram_tensor("woT", [D, D], BF, kind="ExternalInput")
    wbT = nc.dram_tensor("wbT", [T, TH], BF, kind="ExternalInput")
    bqc = nc.dram_tensor("bqc", [P, DT], F32, kind="ExternalInput")
    bvc = nc.dram_tensor("bvc", [P, DT], F32, kind="ExternalInput")
    bob = nc.dram_tensor("bob", [P, D], F32, kind="ExternalInput")
    y = nc.dram_tensor("y", [TH, D], F32, kind="ExternalOutput")

    xT_v = xT.rearrange("(o p) t -> p o t", p=P)
    wq_v = wqT.rearrange("(o p) e -> p o e", p=P)
    wk_v = wkT.rearrange("(o p) e -> p o e", p=P)
    wv_v = wvT.rearrange("(o p) e -> p o e", p=P)
    wo_v = woT.rearrange("(o p) e -> p o e", p=P)
    wb_v = wbT.rearrange("(o p) t -> p o t", p=P)
    y_v = y.rearrange("(o p) e -> p o e", p=P)

    with tile.TileContext(nc) as tc:
        with (
            tc.tile_pool(name="big", bufs=1) as big,
            tc.tile_pool(name="w", bufs=2) as wpool,
            tc.tile_pool(name="tmp", bufs=2) as tmp,
            tc.tile_pool(name="wbst", bufs=4) as wbpool,
            tc.tile_pool(name="bias", bufs=1) as biasp,
            tc.tile_pool(name="ewh0", bufs=1) as ewp0,
            tc.tile_pool(name="psum", bufs=6, space="PSUM") as psum,
        ):
            eK = big.tile([P, ST, D], BF, tag="eK")
            eKV = big.tile([P, ST, D], BF, tag="eKV")
            sigQT = big.tile([P, DT, TH], BF, tag="sigQT")
            outPT = big.tile([P, DT, TH], BF, tag="outPT")

            # ew^T halves: [s, t-chunk] each; half 0 prefetches/exps during
            # phase 1 (its pool does not overlap the x pool), half 1 lands
            # in the space x frees and overlaps the first AFT chunk.
            ewh = [None, None]
            ewh[0] = ewp0.tile([P, ST, CH], BF, tag="ewh0", name="ewh0")

            def emit_ew_half(c, dst):
                for q in range(ST // 2):
                    stg = wbpool.tile([P, 2, CH], BF, tag="wbst")
                    nc.sync.dma_start(
                        stg[:],
                        wb_v[:, 2 * q : 2 * q + 2, c * CH : (c + 1) * CH],
                    )
                    nc.scalar.activation(
                        dst[:, 2 * q : 2 * q + 2, :], stg[:], Act.Exp
                    )

            with tc.tile_pool(name="x", bufs=1) as xpool:
                xs = xpool.tile([P, DT, T], BF, tag="xs")
                wk_s = wpool.tile([P, DT, D], BF, tag="w")
                wv_s = wpool.tile([P, DT, D], BF, tag="w")
                # interleave the input DMAs in consumption order so the
                # first matmuls can start after ~1 MB instead of ~10 MB
                for k in range(DT):
                    nc.sync.dma_start(xs[:, k, :], xT_v[:, k, :])
                    nc.sync.dma_start(wk_s[:, k, :], wk_v[:, k, :])
                emit_ew_half(0, ewh[0])
                for k in range(DT):
                    nc.sync.dma_start(wv_s[:, k, :], wv_v[:, k, :])
                wq_s = wpool.tile([P, DT, D], BF, tag="w")
                for k in range(DT):
                    nc.sync.dma_start(wq_s[:, k, :], wq_v[:, k, :])
                bq_s = biasp.tile([P, DT], F32, tag="bq")
                nc.sync.dma_start(bq_s[:], bqc[:])
                bv_s = biasp.tile([P, DT], F32, tag="bv")
                nc.sync.dma_start(bv_s[:], bvc[:])
                bo_s = biasp.tile([P, D], F32, tag="bo")
                nc.sync.dma_start(bo_s[:], bob[:])

                # ---- K, V projections -> eK = exp(K), eKV = eK * V ----
                for st in range(ST):
                    tsl = slice(st * P, (st + 1) * P)
                    for ec in range(NCH_D):
                        esl = slice(ec * CH, (ec + 1) * CH)
                        pk = psum.tile([P, CH], F32, tag="ps")
                        for k in range(DT):
                            nc.tensor.matmul(
                                pk[:], xs[:, k, tsl], wk_s[:, k, esl],
                                start=(k == 0), stop=(k == DT - 1),
                            )
                        nc.scalar.activation(eK[:, st, esl], pk[:], Act.Exp)
                    for ec in range(NCH_D):
                        esl = slice(ec * CH, (ec + 1) * CH)
                        pv = psum.tile([P, CH], F32, tag="ps")
                        for k in range(DT):
                            nc.tensor.matmul(
                                pv[:], xs[:, k, tsl], wv_s[:, k, esl],
                                start=(k == 0), stop=(k == DT - 1),
                            )
                        nc.vector.tensor_tensor(
                            eKV[:, st, esl], eK[:, st, esl], pv[:], Alu.mult
                        )

                # ---- Q^T projection -> sigQT = sigmoid(Q^T + bq) ----
                # (columns 0:TH of xs are this core's own t rows)
                for et in range(DT):
                    esl = slice(et * P, (et + 1) * P)
                    for c in range(NCH_T):
                        tsl = slice(c * CH, (c + 1) * CH)
                        pq = psum.tile([P, CH], F32, tag="ps")
                        for k in range(DT):
                            nc.tensor.matmul(
                                pq[:], wq_s[:, k, esl], xs[:, k, tsl],
                                start=(k == 0), stop=(k == DT - 1),
                            )
                        nc.scalar.activation(
                            sigQT[:, et, tsl], pq[:], Act.Sigmoid,
                            bias=bq_s[:, et : et + 1],
                        )

            # ---- AFT: numerT/denomT accumulation + ratio, one ew half
            # (= one 512-wide t-chunk) at a time ----
            with tc.tile_pool(name="ewh1", bufs=1) as ewp1:
                ewh[1] = ewp1.tile([P, ST, CH], BF, tag="ewh1", name="ewh1")
                emit_ew_half(1, ewh[1])

                for c in range(NCH_T):
                    tsl = slice(c * CH, (c + 1) * CH)
                    ew_c = ewh[c]
                    for dti in range(DT):
                        dsl = slice(dti * P, (dti + 1) * P)
                        pn = psum.tile([P, CH], F32, tag="ps")
                        pd = psum.tile([P, CH], F32, tag="ps")
                        for ss in range(ST):
                            nc.tensor.matmul(
                                pn[:], eKV[:, ss, dsl], ew_c[:, ss, :],
                                start=(ss == 0), stop=(ss == ST - 1),
                            )
                        for ss in range(ST):
                            nc.tensor.matmul(
                                pd[:], eK[:, ss, dsl], ew_c[:, ss, :],
                                start=(ss == 0), stop=(ss == ST - 1),
                            )
                        rec = tmp.tile([P, CH], F32, tag="rec")
                        nc.vector.reciprocal(rec[:], pd[:])
                        rat = tmp.tile([P, CH], F32, tag="rat")
                        nc.vector.tensor_tensor(rat[:], pn[:], rec[:], Alu.mult)
                        nc.vector.tensor_scalar(
                            rat[:], rat[:], bv_s[:, dti : dti + 1], None, Alu.add
                        )
                        nc.vector.tensor_tensor(
                            outPT[:, dti, tsl], rat[:], sigQT[:, dti, tsl],
                            Alu.mult,
                        )

                # ---- output projection: y = outPT^T @ woT + bo ----
                wo_s = wpool.tile([P, DT, D], BF, tag="w")
                for k in range(DT):
                    nc.sync.dma_start(wo_s[:, k, :], wo_v[:, k, :])
                for tt in range(DT):
                    tsl = slice(tt * P, (tt + 1) * P)
                    for ec in range(NCH_D):
                        esl = slice(ec * CH, (ec + 1) * CH)
                        py = psum.tile([P, CH], F32, tag="ps")
                        for k in range(DT):
                            nc.tensor.matmul(
                                py[:], outPT[:, k, tsl], wo_s[:, k, esl],
                                start=(k == 0), stop=(k == DT - 1),
                            )
                        ysb = tmp.tile([P, CH], F32, tag="ysb")
                        nc.vector.tensor_tensor(
                            ysb[:], py[:], bo_s[:, esl], Alu.add
                        )
                        nc.sync.dma_start(y_v[:, tt, esl], ysb[:])

    nc.compile()
    return nc


def _get_nc():
    if "nc" not in _cache:
        _cache["nc"] = _build_nc()
    return _cache["nc"]


def kernel(x, dummy, Wq, bq, Wk, bk, Wv, bv, Wo, bo, wbias):
    import os

    x = np.asarray(x, np.float32)
    Wq = np.asarray(Wq, np.float32)
    Wk = np.asarray(Wk, np.float32)
    Wv = np.asarray(Wv, np.float32)
    Wo = np.asarray(Wo, np.float32)
    bq = np.asarray(bq, np.float32)
    bv = np.asarray(bv, np.float32)
    bo = np.asarray(bo, np.float32)
    wbias = np.asarray(wbias, np.float32)

    wqT = np.ascontiguousarray(Wq.T).astype(BF16)  # [d_in, e_out]
    wkT = np.ascontiguousarray(Wk.T).astype(BF16)
    wvT = np.ascontiguousarray(Wv.T).astype(BF16)
    woT = np.ascontiguousarray(Wo.T).astype(BF16)
    bqc = np.ascontiguousarray(bq.reshape(DT, P).T)  # [P, DT]
    bvc = np.ascontiguousarray(bv.reshape(DT, P).T)
    bob = np.ascontiguousarray(np.broadcast_to(bo, (P, D)))

    in_maps = []
    for c in range(8):
        b, th = c // 2, c % 2
        t0 = th * TH
        xTb = x[b].T  # [D, T]
        # roll the s (contraction) axis so this core's own t rows come
        # first; Q then always reads columns [0, TH).
        xT_in = np.concatenate([xTb[:, t0:], xTb[:, :t0]], axis=1)
        wbn = wbias[t0 : t0 + TH, :].T  # [s, t] natural s order
        wb_in = np.concatenate([wbn[t0:, :], wbn[:t0, :]], axis=0)
        in_maps.append(
            {
                "xT": np.ascontiguousarray(xT_in).astype(BF16),
                "wqT": wqT,
                "wkT": wkT,
                "wvT": wvT,
                "woT": woT,
                "wbT": np.ascontiguousarray(wb_in).astype(BF16),
                "bqc": bqc,
                "bvc": bvc,
                "bob": bob,
            }
        )

    from concourse.bass_utils import run_bass_kernel_spmd

    nc = _get_nc()
    trace = bool(os.environ.get("AFT_TRACE"))
    res = run_bass_kernel_spmd(
        nc, in_maps, core_ids=list(range(8)), trace=trace
    )
    kernel._last_exec_ns = res.exec_time_ns
    kernel._last_result = res

    out = np.empty((B, T, D), np.float32)
    for c in range(8):
        b, th = c // 2, c % 2
        out[b, th * TH : (th + 1) * TH, :] = res.results[c]["y"]
    return out


# revision 17
# speedup vs baseline: 1.2613x; 1.2265x over previous
"""AFT-Full (Attention Free Transformer) forward on 8 Trainium2 NeuronCores.

Reference computation (B=4, T=2048, D=1024, H=16, dh=64):
    Q = x @ Wq.T + bq ; K = x @ Wk.T + bk ; V = x @ Wv.T + bv
    ew = exp(wbias)                       # [T, T]
    numer = ew @ (exp(K) * V)             # per (b, h)
    denom = ew @ exp(K)
    out = sigmoid(Q) * numer / denom
    y = out @ Wo.T + bo

Sharding: 8 cores = 4 batches x 2 output-row halves. Core c handles batch
b = c//2 and output rows [th*1024, th*1024+1024) with th = c%2. K/V are
(re)computed for the full sequence on both cores of a batch pair; no
cross-core communication is needed at all.

Identities used to avoid broadcast-bias work on device:
  - bk cancels exactly in numer/denom (exp(K+bk) = exp(K)*exp(bk), and the
    exp(bk[d]) factor is constant over the contraction index s).
  - bv shifts the ratio: ew@(eK*(V+bv)) / ew@eK = ew@(eK*V)/ew@eK + bv[d],
    applied as a per-partition scalar add in the [d, t] layout.
  - bq is a per-partition bias in the Q^T [e, t] layout, fused into the
    sigmoid activation. bo is added during PSUM evacuation from a
    host-broadcast [128, D] tile.

All matmuls run in bf16 (fp32 PSUM accumulation). The host pre-transposes
x, the weights and wbias so the device needs no transposes, and rolls the
contraction (s) axis by the core's t-offset so a single shared program
serves both t-halves.
"""

import sys

if "/opt/trn_rl_repo" not in sys.path:
    sys.path.insert(0, "/opt/trn_rl_repo")

import numpy as np
import ml_dtypes

BF16 = ml_dtypes.bfloat16

B, T, D = 4, 2048, 1024
TH = T // 2  # rows per core
P = 128
CH = 512  # psum chunk (one fp32 bank)
DT = D // P  # 8   d-tiles
ST = T // P  # 16  s-tiles
NCH_D = D // CH  # 2
NCH_T = TH // CH  # 2

_cache = {}


def _build_nc():
    import concourse.mybir as mybir
    import concourse.tile as tile
    from concourse import bacc

    dt = mybir.dt
    BF = dt.bfloat16
    F32 = dt.float32
    Act = mybir.ActivationFunctionType
    Alu = mybir.AluOpType

    nc = bacc.Bacc("TRN2")

    xT = nc.dram_tensor("xT", [D, T], BF, kind="ExternalInput")
    wqT = nc.dram_tensor("wqT", [D, D], BF, kind="ExternalInput")
    wkT = nc.dram_tensor("wkT", [D, D], BF, kind="ExternalInput")
    wvT = nc.dram_tensor("wvT", [D, D], BF, kind="ExternalInput")
    woT = nc.dram_tensor("woT", [D, D], BF, kind="ExternalInput")
    wbT = nc.dram_tensor("wbT", [T, TH], BF, kind="ExternalInput")
    bqc = nc.dram_tensor("bqc", [P, DT], F32, kind="ExternalInput")
    bvc = nc.dram_tensor("bvc", [P, DT], F32, kind="ExternalInput")
    bob = nc.dram_tensor("bob", [P, D], F32, kind="ExternalInput")
    y = nc.dram_tensor("y", [TH, D], F32, kind="ExternalOutput")

    xT_v = xT.rearrange("(o p) t -> p o t", p=P)
    wq_v = wqT.rearrange("(o p) e -> p o e", p=P)
    wk_v = wkT.rearrange("(o p) e -> p o e", p=P)
    wv_v = wvT.rearrange("(o p) e -> p o e", p=P)
    wo_v = woT.rearrange("(o p) e -> p o e", p=P)
    wb_v = wbT.rearrange("(o p) t -> p o t", p=P)
    y_v = y.rearrange("(o p) e -> p o e", p=P)

    with tile.TileContext(nc) as tc:
        with (
            tc.tile_pool(name="big", bufs=1) as big,
            tc.tile_pool(name="w", bufs=2) as wpool,
            tc.tile_pool(name="tmp", bufs=3) as tmp,
            tc.tile_pool(name="wbst", bufs=4) as wbpool,
            tc.tile_pool(name="bias", bufs=1) as biasp,
            tc.tile_pool(name="ewh0", bufs=1) as ewp0,
            tc.tile_pool(name="psum", bufs=8, space="PSUM") as psum,
        ):
            eK = big.tile([P, ST, D], BF, tag="eK")
            eKV = big.tile([P, ST, D], BF, tag="eKV")
            sigQT = big.tile([P, DT, TH], BF, tag="sigQT")
            outPT = big.tile([P, DT, TH], BF, tag="outPT")

            # ew^T halves: [s, t-chunk] each; half 0 prefetches/exps during
            # phase 1 (its pool does not overlap the x pool), half 1 lands
            # in the space x frees and overlaps the first AFT chunk.
            ewh = [None, None]
            ewh[0] = ewp0.tile([P, ST, CH], BF, tag="ewh0", name="ewh0")

            def emit_ew_half(c, dst):
                for q in range(ST // 2):
                    stg = wbpool.tile([P, 2, CH], BF, tag="wbst")
                    nc.sync.dma_start(
                        stg[:],
                        wb_v[:, 2 * q : 2 * q + 2, c * CH : (c + 1) * CH],
                    )
                    nc.scalar.activation(
                        dst[:, 2 * q : 2 * q + 2, :], stg[:], Act.Exp
                    )

            with tc.tile_pool(name="x", bufs=1) as xpool:
                xs = xpool.tile([P, DT, T], BF, tag="xs")
                wk_s = wpool.tile([P, DT, D], BF, tag="w")
                wv_s = wpool.tile([P, DT, D], BF, tag="w")
                # PE warm-up: ~12 throwaway matmuls on a memset tile run
                # during the initial input-DMA wait, so the HAM clock gate
                # is already at 2.4 GHz when the first real matmul issues.
                warm = biasp.tile([P, CH], BF, tag="warm")
                nc.vector.memset(warm[:], 0.0)
                pwarm = psum.tile([P, CH], F32, tag="ps", name="pwarm")
                for _ in range(6):
                    nc.tensor.matmul(
                        pwarm[:], warm[:, :P], warm[:],
                        start=True, stop=True,
                    )

                # interleave the input DMAs in consumption order so the
                # first matmuls can start after ~1 MB instead of ~10 MB;
                # the k=0 slices are split again so the very first matmul
                # only waits for ~160 KB
                nc.sync.dma_start(xs[:, 0, :P], xT_v[:, 0, :P])
                nc.sync.dma_start(wk_s[:, 0, :CH], wk_v[:, 0, :CH])
                nc.sync.dma_start(xs[:, 0, P:], xT_v[:, 0, P:])
                nc.sync.dma_start(wk_s[:, 0, CH:], wk_v[:, 0, CH:])
                for k in range(1, DT):
                    nc.sync.dma_start(xs[:, k, :], xT_v[:, k, :])
                    nc.sync.dma_start(wk_s[:, k, :], wk_v[:, k, :])
                for k in range(DT):
                    nc.sync.dma_start(wv_s[:, k, :], wv_v[:, k, :])
                emit_ew_half(0, ewh[0])
                wq_s = wpool.tile([P, DT, D], BF, tag="w")
                for k in range(DT):
                    nc.sync.dma_start(wq_s[:, k, :], wq_v[:, k, :])
                bq_s = biasp.tile([P, DT], F32, tag="bq")
                nc.sync.dma_start(bq_s[:], bqc[:])
                bv_s = biasp.tile([P, DT], F32, tag="bv")
                nc.sync.dma_start(bv_s[:], bvc[:])
                bo_s = biasp.tile([P, D], F32, tag="bo")
                nc.sync.dma_start(bo_s[:], bob[:])

                # ---- K projection -> eK = exp(K) ----
                # k-OUTER emission in groups of 8 PSUM tiles: the PE stream
                # consumes x/w k-slices in DMA-arrival order, so the engine
                # is never blocked behind a not-yet-arrived slice during the
                # input ramp (PE executes its stream strictly in order).
                for g in range(2):
                    tiles = [
                        (g * 4 + i, ec) for i in range(4) for ec in range(NCH_D)
                    ]
                    pks = {}
                    for (st, ec) in tiles:
                        pks[(st, ec)] = psum.tile(
                            [P, CH], F32, tag="ps", name=f"pk_{st}_{ec}"
                        )
                    for k in range(DT):
                        for (st, ec) in tiles:
                            nc.tensor.matmul(
                                pks[(st, ec)][:],
                                xs[:, k, st * P : (st + 1) * P],
                                wk_s[:, k, ec * CH : (ec + 1) * CH],
                                start=(k == 0), stop=(k == DT - 1),
                            )
                    for (st, ec) in tiles:
                        nc.scalar.activation(
                            eK[:, st, ec * CH : (ec + 1) * CH],
                            pks[(st, ec)][:], Act.Exp,
                        )
                # post-ramp: all slices resident, tile-outer keeps the
                # 8 PSUM slots rotating one at a time (no group barrier)
                for st in range(8, ST):
                    tsl = slice(st * P, (st + 1) * P)
                    for ec in range(NCH_D):
                        esl = slice(ec * CH, (ec + 1) * CH)
                        pk = psum.tile([P, CH], F32, tag="ps")
                        for k in range(DT):
                            nc.tensor.matmul(
                                pk[:], xs[:, k, tsl], wk_s[:, k, esl],
                                start=(k == 0), stop=(k == DT - 1),
                            )
                        nc.scalar.activation(eK[:, st, esl], pk[:], Act.Exp)

                # ---- V projection -> eKV = eK * V ----
                for st in range(ST):
                    tsl = slice(st * P, (st + 1) * P)
                    for ec in range(NCH_D):
                        esl = slice(ec * CH, (ec + 1) * CH)
                        pv = psum.tile([P, CH], F32, tag="ps")
                        for k in range(DT):
                            nc.tensor.matmul(
                                pv[:], xs[:, k, tsl], wv_s[:, k, esl],
                                start=(k == 0), stop=(k == DT - 1),
                            )
                        nc.vector.tensor_tensor(
                            eKV[:, st, esl], eK[:, st, esl], pv[:], Alu.mult
                        )

                # ---- Q^T projection -> sigQT = sigmoid(Q^T + bq) ----
                # (columns 0:TH of xs are this core's own t rows)
                for et in range(DT):
                    esl = slice(et * P, (et + 1) * P)
                    for c in range(NCH_T):
                        tsl = slice(c * CH, (c + 1) * CH)
                        pq = psum.tile([P, CH], F32, tag="ps")
                        for k in range(DT):
                            nc.tensor.matmul(
                                pq[:], wq_s[:, k, esl], xs[:, k, tsl],
                                start=(k == 0), stop=(k == DT - 1),
                            )
                        nc.scalar.activation(
                            sigQT[:, et, tsl], pq[:], Act.Sigmoid,
                            bias=bq_s[:, et : et + 1],
                        )

            # ---- AFT: numerT/denomT accumulation + ratio, one ew half
            # (= one 512-wide t-chunk) at a time ----
            with tc.tile_pool(name="ewh1", bufs=1) as ewp1:
                ewh[1] = ewp1.tile([P, ST, CH], BF, tag="ewh1", name="ewh1")
                emit_ew_half(1, ewh[1])

                for c in range(NCH_T):
                    tsl = slice(c * CH, (c + 1) * CH)
                    ew_c = ewh[c]
                    for dti in range(DT):
                        dsl = slice(dti * P, (dti + 1) * P)
                        pn = psum.tile([P, CH], F32, tag="ps")
                        pd = psum.tile([P, CH], F32, tag="ps")
                        for ss in range(ST):
                            nc.tensor.matmul(
                                pn[:], eKV[:, ss, dsl], ew_c[:, ss, :],
                                start=(ss == 0), stop=(ss == ST - 1),
                            )
                        for ss in range(ST):
                            nc.tensor.matmul(
                                pd[:], eK[:, ss, dsl], ew_c[:, ss, :],
                                start=(ss == 0), stop=(ss == ST - 1),
                            )
                        rec = tmp.tile([P, CH], F32, tag="rec")
                        nc.vector.reciprocal_approx_fast(rec[:], pd[:])
                        rat = tmp.tile([P, CH], F32, tag="rat")
                        nc.vector.tensor_tensor(rat[:], pn[:], rec[:], Alu.mult)
                        nc.vector.tensor_scalar(
                            rat[:], rat[:], bv_s[:, dti : dti + 1], None, Alu.add
                        )
                        nc.vector.tensor_tensor(
                            outPT[:, dti, tsl], rat[:], sigQT[:, dti, tsl],
                            Alu.mult,
                        )

                # ---- output projection: y = outPT^T @ woT + bo ----
                wo_s = wpool.tile([P, DT, D], BF, tag="w")
                for k in range(DT):
                    nc.sync.dma_start(wo_s[:, k, :], wo_v[:, k, :])
                for tt in range(DT):
                    tsl = slice(tt * P, (tt + 1) * P)
                    for ec in range(NCH_D):
                        esl = slice(ec * CH, (ec + 1) * CH)
                        py = psum.tile([P, CH], F32, tag="ps")
                        for k in range(DT):
                            nc.tensor.matmul(
                                py[:], outPT[:, k, tsl], wo_s[:, k, esl],
                                start=(k == 0), stop=(k == DT - 1),
                            )
                        ysb = tmp.tile([P, CH], F32, tag="ysb")
                        nc.vector.tensor_tensor(
                            ysb[:], py[:], bo_s[:, esl], Alu.add
                        )
                        nc.sync.dma_start(y_v[:, tt, esl], ysb[:])

    nc.compile()
    return nc


def _get_nc():
    if "nc" not in _cache:
        _cache["nc"] = _build_nc()
    return _cache["nc"]


def kernel(x, dummy, Wq, bq, Wk, bk, Wv, bv, Wo, bo, wbias):
    import os

    x = np.asarray(x, np.float32)
    Wq = np.asarray(Wq, np.float32)
    Wk = np.asarray(Wk, np.float32)
    Wv = np.asarray(Wv, np.float32)
    Wo = np.asarray(Wo, np.float32)
    bq = np.asarray(bq, np.float32)
    bv = np.asarray(bv, np.float32)
    bo = np.asarray(bo, np.float32)
    wbias = np.asarray(wbias, np.float32)

    wqT = np.ascontiguousarray(Wq.T).astype(BF16)  # [d_in, e_out]
    wkT = np.ascontiguousarray(Wk.T).astype(BF16)
    wvT = np.ascontiguousarray(Wv.T).astype(BF16)
    woT = np.ascontiguousarray(Wo.T).astype(BF16)
    bqc = np.ascontiguousarray(bq.reshape(DT, P).T)  # [P, DT]
    bvc = np.ascontiguousarray(bv.reshape(DT, P).T)
    bob = np.ascontiguousarray(np.broadcast_to(bo, (P, D)))

    in_maps = []
    for c in range(8):
        b, th = c // 2, c % 2
        t0 = th * TH
        xTb = x[b].T  # [D, T]
        # roll the s (contraction) axis so this core's own t rows come
        # first; Q then always reads columns [0, TH).
        xT_in = np.concatenate([xTb[:, t0:], xTb[:, :t0]], axis=1)
        wbn = wbias[t0 : t0 + TH, :].T  # [s, t] natural s order
        wb_in = np.concatenate([wbn[t0:, :], wbn[:t0, :]], axis=0)
        in_maps.append(
            {
                "xT": np.ascontiguousarray(xT_in).astype(BF16),
                "wqT": wqT,
                "wkT": wkT,
                "wvT": wvT,
                "woT": woT,
                "wbT": np.ascontiguousarray(wb_in).astype(BF16),
                "bqc": bqc,
                "bvc": bvc,
                "bob": bob,
            }
        )

    from concourse.bass_utils import run_bass_kernel_spmd

    nc = _get_nc()
    trace = bool(os.environ.get("AFT_TRACE"))
    res = run_bass_kernel_spmd(
        nc, in_maps, core_ids=list(range(8)), trace=trace
    )
    kernel._last_exec_ns = res.exec_time_ns
    kernel._last_result = res

    out = np.empty((B, T, D), np.float32)
    for c in range(8):
        b, th = c // 2, c % 2
        out[b, th * TH : (th + 1) * TH, :] = res.results[c]["y"]
    return out
